# revision 1
# baseline (speedup 1.0000x reference)
"""CPDecoding (embedding_lookup) Trainium2 kernel.

out[n] = sum_c fz[c,n]*fy[c,n]*fx[c,n], where f* is a 1-D linear
interpolation (grid_sample, align_corners=True) of a (96, 512) line table
at per-point coordinates.

Strategy (8 cores, data-parallel over the N=4096*192 points):
  - Host: shard points, pre-permute layouts, pack tables as fp16
    [f0 | delta | pad] rows of 512B (one row per table position i holds
    L[:, i] and L[:, i+1]-L[:, i]).
  - Device: per-point (i0, w) on DVE; per-chunk dma_gather of one 512B row
    per (point, axis) from HBM; DVE interp f0 + w*delta, 3-way product,
    free-axis reduce over the 96 components. ~571us/core (cost model),
    memory-bound-adjacent: 151MB of gathered rows per core.
"""

import numpy as np

N_CORES = 8
N_TOTAL = 4096 * 192
N_CORE = N_TOTAL // N_CORES      # 98304 points per core
P = 128                          # partitions
F = N_CORE // P                  # 768 free blocks (wrapped-128 layout)
GROUPS = 8                       # wrapped-16 partition groups
PHI = N_CORE // 16 // GROUPS     # 768 phi-columns per group
C = 96                           # components
R = 512                          # table resolution
ELEM = 256                       # fp16 elements per table row (512 bytes)
CHUNK_F = 32                     # f-blocks per chunk
CHUNK_PTS = P * CHUNK_F          # 4096 points per chunk
N_CHUNKS = F // CHUNK_F          # 24
CHUNKS_PER_GROUP = N_CHUNKS // GROUPS  # 3
SUBCOLS = CHUNK_PTS // 16        # 256 idx columns per chunk

# axis a -> coordinate column in in_tensor (reference samples [z, y, x] from
# pts[:, 2], pts[:, 1], pts[:, 0])
AXIS_COL = [2, 1, 0]

_BUILT = None


def _build_nc():
    """Build the per-core Bass program (SPMD, identical on all cores)."""
    import concourse.bacc as bacc
    import concourse.bass as bass
    import concourse.tile as tile
    from concourse import mybir
    from concourse.library_config import mlp as lib_mlp

    dt = mybir.dt
    Alu = mybir.AluOpType
    Axis = mybir.AxisListType

    nc = bacc.Bacc("TRN2", target_bir_lowering=False, debug=False,
                   num_devices=N_CORES, num_swdge_queues=1)

    pw128 = nc.dram_tensor("pw128", [P, F * 3], dt.float32,
                           kind="ExternalInput").ap()
    pw16 = nc.dram_tensor("pw16", [P, PHI * 3], dt.float32,
                          kind="ExternalInput").ap()
    tbls = [nc.dram_tensor(f"tbl{a}", [R, ELEM], dt.float16,
                           kind="ExternalInput").ap() for a in range(3)]
    out_d = nc.dram_tensor("out", [P, F], dt.float32,
                           kind="ExternalOutput").ap()

    with tile.TileContext(nc) as tc:
        with tc.tile_pool(name="persist", bufs=1) as pp:
            # persistent tiles
            w_axis = [pp.tile([P, F], dt.float16, tag=f"w{a}",
                              name=f"w_axis{a}") for a in range(3)]
            idx_all = pp.tile([P, 3, PHI], dt.int16, tag="idx")
            out_full = pp.tile([P, F], dt.float32, tag="out")

            # ---------- setup: per-point index/weight math ----------
            with tc.tile_pool(name="setup", bufs=1) as sp:
                s128 = sp.tile([P, F * 3], dt.float32, tag="c0")
                nc.sync.dma_start(s128[:], pw128)
                s16 = sp.tile([P, PHI * 3], dt.float32, tag="c1")
                nc.sync.dma_start(s16[:], pw16)

                def idx_chain(src, n_free, want_w, tg):
                    def tmp(nm, dtype=dt.float32):
                        return sp.tile([P, n_free], dtype,
                                       tag="tmp", bufs=4, name=f"{nm}{tg}")
                    # pos = ((coord + 1) * 0.5) * 511, matching jax op order
                    t1 = tmp("t1")
                    nc.vector.tensor_scalar(t1[:], src[:], 1.0, 0.5,
                                            Alu.add, Alu.mult)
                    pos = tmp("pos")
                    nc.vector.tensor_scalar(pos[:], t1[:], 511.0, None,
                                            Alu.mult)
                    # floor(pos) via int round-trip; the fp->int cast may
                    # truncate or round-to-nearest, so fix up with a compare
                    ii = tmp("ii", dt.int32)
                    nc.vector.tensor_copy(ii[:], pos[:])
                    i0f = tmp("i0f")
                    nc.vector.tensor_copy(i0f[:], ii[:])
                    neg = tmp("neg")
                    nc.vector.tensor_tensor(neg[:], pos[:], i0f[:], Alu.is_lt)
                    i0a = tmp("i0a")
                    nc.vector.tensor_sub(i0a[:], i0f[:], neg[:])
                    i0c = tmp("i0c")
                    nc.vector.tensor_scalar(i0c[:], i0a[:], 510.0, 0.0,
                                            Alu.min, Alu.max)
                    if not want_w:
                        return i0c, None
                    w = tmp("w")
                    nc.vector.tensor_sub(w[:], pos[:], i0c[:])
                    return i0c, w

                _, w128 = idx_chain(s128, F * 3, True, "a")
                w128_3d = w128[:].rearrange("p (f k) -> p f k", k=3)
                for a in range(3):
                    nc.vector.tensor_copy(w_axis[a][:],
                                          w128_3d[:, :, AXIS_COL[a]])

                i0c16, _ = idx_chain(s16, PHI * 3, False, "b")
                i16_3d = i0c16[:].rearrange("p (f k) -> p f k", k=3)
                for a in range(3):
                    nc.vector.tensor_copy(idx_all[:, a, :],
                                          i16_3d[:, :, AXIS_COL[a]])

            # ---------- main loop ----------
            with (
                tc.tile_pool(name="stg", bufs=3) as stg_pool,
                tc.tile_pool(name="gath", bufs=2) as gath_pool,
                tc.tile_pool(name="mid", bufs=2) as mid_pool,
            ):
                with tc.tile_critical():
                    nc.gpsimd.load_library(lib_mlp)
                stg_tiles = {}
                for c in range(N_CHUNKS):
                    grp = c // CHUNKS_PER_GROUP
                    sub = c % CHUNKS_PER_GROUP

                    if sub == 0:
                        # stage group grp's indices, replicated into every
                        # 16-partition band (the SWDGE queue's core pair
                        # reads its own 32-partition window)
                        stg = stg_pool.tile([P, 3, PHI], dt.int16, tag="stg")
                        src = idx_all[16 * grp:16 * (grp + 1), :, :]
                        for b in range(8):
                            nc.sync.dma_start(
                                stg[16 * b:16 * (b + 1), :, :], src)
                        stg_tiles[grp] = stg
                    stg = stg_tiles[grp]

                    az = []
                    for a in range(3):
                        g = gath_pool.tile([P, CHUNK_F, ELEM], dt.float16,
                                           tag=f"g{a}")
                        idxs = stg[:, a, SUBCOLS * sub:SUBCOLS * (sub + 1)]
                        nc.gpsimd.dma_gather(
                            g[:], tbls[a], idxs, CHUNK_PTS, CHUNK_PTS, ELEM,
                            elem_step=ELEM, queue_num=0, single_packet=False)

                        f0 = g[:, :, 0:C]
                        dd = g[:, :, C:2 * C]
                        wb = (w_axis[a][:, CHUNK_F * c:CHUNK_F * (c + 1)]
                              .unsqueeze(2).broadcast_to([P, CHUNK_F, C]))
                        u = mid_pool.tile([P, CHUNK_F, C], dt.float16,
                                          tag="u")
                        nc.vector.tensor_mul(u[:], dd, wb)
                        azt = mid_pool.tile([P, CHUNK_F, C], dt.float16,
                                            tag=f"az{a}")
                        nc.vector.tensor_add(azt[:], f0, u[:])
                        az.append(azt)

                    p = mid_pool.tile([P, CHUNK_F, C], dt.float16, tag="p")
                    nc.vector.tensor_mul(p[:], az[0][:], az[1][:])
                    qq = mid_pool.tile([P, CHUNK_F, C], dt.float16, tag="q")
                    nc.vector.tensor_mul(qq[:], p[:], az[2][:])
                    nc.vector.reduce_sum(
                        out_full[:, CHUNK_F * c:CHUNK_F * (c + 1)],
                        qq[:], axis=Axis.X)

                nc.sync.dma_start(out_d, out_full[:])

    nc.compile()
    return nc


def _host_prep(in_tensor, line_z, line_y, line_x):
    """Build per-core input maps (layout permutations + table packing)."""
    pts = np.ascontiguousarray(in_tensor.reshape(-1, 3).astype(np.float32))

    tables = []
    for L in (line_z, line_y, line_x):
        Lf = np.asarray(L, dtype=np.float32)
        f0 = Lf.T                                    # (512, 96)
        f1 = np.concatenate([Lf.T[1:], Lf.T[-1:]], axis=0)
        row = np.zeros((R, ELEM), dtype=np.float16)
        row[:, 0:C] = f0.astype(np.float16)
        row[:, C:2 * C] = (f1 - f0).astype(np.float16)
        tables.append(row)

    in_maps = []
    for k in range(N_CORES):
        shard = pts[k * N_CORE:(k + 1) * N_CORE]
        pw128 = np.ascontiguousarray(
            shard.reshape(F, P, 3).transpose(1, 0, 2).reshape(P, F * 3))
        pw16 = np.ascontiguousarray(
            shard.reshape(GROUPS, PHI, 16, 3).transpose(0, 2, 1, 3)
            .reshape(P, PHI * 3))
        in_maps.append({
            "pw128": pw128,
            "pw16": pw16,
            "tbl0": tables[0],
            "tbl1": tables[1],
            "tbl2": tables[2],
        })
    return in_maps


def _unshard(results):
    outs = []
    for k in range(N_CORES):
        w = np.asarray(results[k]["out"])            # (128, 768), n = 128f + p
        outs.append(w.T.reshape(-1))
    return np.concatenate(outs).reshape(4096, 192).astype(np.float32)


def kernel(in_tensor, line_z, line_y, line_x):
    global _BUILT
    from concourse.bass_utils import run_bass_kernel_spmd

    if _BUILT is None:
        _BUILT = _build_nc()
    nc = _BUILT
    in_maps = _host_prep(np.asarray(in_tensor), np.asarray(line_z),
                         np.asarray(line_y), np.asarray(line_x))
    res = run_bass_kernel_spmd(nc, in_maps, list(range(N_CORES)))
    return _unshard(res.results)



# revision 8
# speedup vs baseline: 1.4157x; 1.4157x over previous
"""CPDecoding (embedding_lookup) Trainium2 kernel, v2.

out[n] = sum_c fz[c,n]*fy[c,n]*fx[c,n], where f* is a 1-D linear
interpolation (grid_sample, align_corners=True) of a (96, 512) line table
at per-point coordinates.

Strategy (8 cores, data-parallel over the N=4096*192 points):
  - Host: compute (i0, w) per point/axis, sort points by z-index and pack
    8 points per z-table row (one 512B gather descriptor serves 8 points),
    pad to a fixed slot count; pack tables as fp16 [f0|delta|pad] 512B rows.
  - Device: dma_gather of one row per (point, y/x-axis) and one row per
    8-point z-group; Act engine materializes per-point weights broadcast
    over components; DVE does interp + products + component reduce; Pool
    does one interp mul + SWDGE descriptor generation.
  - Host: unpermute the per-point sums back to the original order.
"""

import numpy as np

N_CORES = 8
N_TOTAL = 4096 * 192
N_CORE = N_TOTAL // N_CORES      # 98304 points per core
P = 128                          # partitions
G = 8                            # points per z-row group
F = 800                          # free slots per partition (padded)
S = P * F                        # 102400 padded point slots per core
C = 96                           # components
R = 512                          # table resolution
ELEM = 256                       # fp16 elements per table row (512 bytes)
CHUNK_F = 32                     # f-blocks per chunk
CHUNK_PTS = P * CHUNK_F          # 4096 points per chunk
N_CHUNKS = F // CHUNK_F          # 25
ZBLK = CHUNK_F // G              # z-row blocks per chunk (4)

_BUILT = None


def _build_nc():
    """Build the per-core Bass program (SPMD, identical on all cores)."""
    import concourse.bacc as bacc
    import concourse.tile as tile
    from concourse import mybir
    from concourse.library_config import mlp as lib_mlp

    dt = mybir.dt
    Alu = mybir.AluOpType
    Axis = mybir.AxisListType

    nc = bacc.Bacc("TRN2", target_bir_lowering=False, debug=False,
                   num_devices=N_CORES, num_swdge_queues=1)

    # host-prepared inputs
    w_d = nc.dram_tensor("w", [P, 3 * F], dt.float16, kind="ExternalInput").ap()
    idxz_d = nc.dram_tensor("idxz", [P, F], dt.int16, kind="ExternalInput").ap()
    idxy_d = nc.dram_tensor("idxy", [P, F * 8], dt.int16,
                            kind="ExternalInput").ap()
    idxx_d = nc.dram_tensor("idxx", [P, F * 8], dt.int16,
                            kind="ExternalInput").ap()
    tbls = [nc.dram_tensor(f"tbl{a}", [R, ELEM], dt.float16,
                           kind="ExternalInput").ap() for a in range(3)]
    out_d = nc.dram_tensor("out", [P, F], dt.float32, kind="ExternalOutput").ap()

    with tile.TileContext(nc) as tc:
        with tc.tile_pool(name="persist", bufs=1) as pp:
            w_all = pp.tile([P, 3, F], dt.float16, tag="w")
            nc.sync.dma_start(w_all[:], w_d)
            idx_z = pp.tile([P, F], dt.int16, tag="iz")
            nc.sync.dma_start(idx_z[:], idxz_d)
            idx_y = pp.tile([P, F * 8], dt.int16, tag="iy")
            nc.sync.dma_start(idx_y[:], idxy_d)
            idx_x = pp.tile([P, F * 8], dt.int16, tag="ix")
            nc.sync.dma_start(idx_x[:], idxx_d)
            out_full = pp.tile([P, F], dt.float32, tag="out")

            with (
                tc.tile_pool(name="gath", bufs=2) as gp,
                tc.tile_pool(name="wt", bufs=2) as wp,
                tc.tile_pool(name="mid", bufs=2) as mp,
            ):
                with tc.tile_critical():
                    nc.gpsimd.load_library(lib_mlp)
                for c in range(N_CHUNKS):
                    fs = CHUNK_F * c

                    # --- gathers (rows: [f0(96) | delta(96) | pad]) ---
                    # idx tiles are wrapped-16 ([16, n/16] per band, replicated
                    # to all 8 bands); chunk c uses its 16-wrapped column slice
                    zc = ZBLK * P // 16              # 32 idx cols per chunk
                    gz = gp.tile([P, ZBLK, ELEM], dt.float16, tag="gz")
                    nc.gpsimd.dma_gather(
                        gz[:], tbls[0], idx_z[:, zc * c:zc * (c + 1)],
                        ZBLK * P, ZBLK * P, ELEM, elem_step=ELEM,
                        queue_num=0, single_packet=False)
                    yc = CHUNK_PTS // 16             # 256 idx cols per chunk
                    gy = gp.tile([P, CHUNK_F, ELEM], dt.float16, tag="gy")
                    nc.gpsimd.dma_gather(
                        gy[:], tbls[1], idx_y[:, yc * c:yc * (c + 1)],
                        CHUNK_PTS, CHUNK_PTS, ELEM, elem_step=ELEM,
                        queue_num=0, single_packet=False)
                    gx = gp.tile([P, CHUNK_F, ELEM], dt.float16, tag="gx")
                    nc.gpsimd.dma_gather(
                        gx[:], tbls[2], idx_x[:, yc * c:yc * (c + 1)],
                        CHUNK_PTS, CHUNK_PTS, ELEM, elem_step=ELEM,
                        queue_num=0, single_packet=False)

                    # --- weight broadcast tiles (Act engine) ---
                    wts = []
                    for a in range(3):
                        wt = wp.tile([P, CHUNK_F, C], dt.float16, tag=f"wt{a}",
                                     name=f"wt{a}")
                        src = (w_all[:, a, fs:fs + CHUNK_F]
                               .unsqueeze(2).broadcast_to([P, CHUNK_F, C]))
                        nc.scalar.copy(wt[:], src)
                        wts.append(wt)

                    # --- z interp (rows shared by groups of 8 points) ---
                    d_z = (gz[:, :, C:2 * C].unsqueeze(2)
                           .broadcast_to([P, ZBLK, G, C]))
                    f0_z = (gz[:, :, 0:C].unsqueeze(2)
                            .broadcast_to([P, ZBLK, G, C]))
                    fz = mp.tile([P, CHUNK_F, C], dt.float16, tag="fz")
                    fz4 = fz[:].rearrange("p (q g) v -> p q g v", g=G)
                    wt04 = wts[0][:].rearrange("p (q g) v -> p q g v", g=G)
                    nc.vector.tensor_mul(fz4, d_z, wt04)
                    nc.vector.tensor_add(fz4, fz4, f0_z)

                    # --- y interp (DVE) ---
                    fy = mp.tile([P, CHUNK_F, C], dt.float16, tag="fy")
                    nc.vector.tensor_mul(fy[:], gy[:, :, C:2 * C], wts[1][:])
                    nc.vector.tensor_add(fy[:], fy[:], gy[:, :, 0:C])

                    # --- x interp (DVE) ---
                    fx = mp.tile([P, CHUNK_F, C], dt.float16, tag="fx")
                    nc.vector.tensor_mul(fx[:], gx[:, :, C:2 * C], wts[2][:])
                    nc.vector.tensor_add(fx[:], fx[:], gx[:, :, 0:C])

                    # --- products + component reduce (DVE) ---
                    nc.vector.tensor_mul(fz[:], fz[:], fy[:])
                    nc.vector.tensor_mul(fz[:], fz[:], fx[:])
                    # binary-tree halvings at tensor_tensor 2x rate, then a
                    # short tensor_reduce tail (reduce gets no DVE perf mode)
                    half = C
                    while half >= 12:
                        half //= 2
                        nc.vector.tensor_add(fz[:, :, 0:half],
                                             fz[:, :, 0:half],
                                             fz[:, :, half:2 * half])
                    nc.vector.reduce_sum(out_full[:, fs:fs + CHUNK_F],
                                         fz[:, :, 0:half], axis=Axis.X)

                nc.sync.dma_start(out_d, out_full[:])

    nc.compile()
    return nc


def _host_prep(in_tensor, line_z, line_y, line_x):
    """Build per-core input maps; returns (in_maps, per-core unsort perms)."""
    pts = np.ascontiguousarray(in_tensor.reshape(-1, 3).astype(np.float32))

    tables = []
    for L in (line_z, line_y, line_x):
        Lf = np.asarray(L, dtype=np.float32)
        f0 = Lf.T                                    # (512, 96)
        f1 = np.concatenate([Lf.T[1:], Lf.T[-1:]], axis=0)
        row = np.zeros((R, ELEM), dtype=np.float16)
        row[:, 0:C] = f0.astype(np.float16)
        row[:, C:2 * C] = (f1 - f0).astype(np.float16)
        tables.append(row)

    # per-point indices/weights, axes ordered [z, y, x] = cols [2, 1, 0]
    pos = (pts + 1.0) * 0.5 * (R - 1)
    i0 = np.clip(np.floor(pos), 0, R - 1).astype(np.int32)
    w = (pos - i0).astype(np.float16)

    def wrap16(flat):
        """j-ordered descriptor index list -> [16, n/16] band, replicated
        to all 8 16-partition bands."""
        w16 = flat.reshape(-1, 16).T
        return np.ascontiguousarray(np.tile(w16, (8, 1)))

    in_maps = []
    perms = []
    for k in range(N_CORES):
        sl = slice(k * N_CORE, (k + 1) * N_CORE)
        iz, iy, ix = i0[sl, 2], i0[sl, 1], i0[sl, 0]
        wz, wy, wx = w[sl, 2], w[sl, 1], w[sl, 0]

        # sort by z-index; emit fixed-size groups of G per z-bin (padded)
        order = np.argsort(iz, kind="stable")
        izs = iz[order]
        # position of each sorted point within its z-bin
        binpos = np.arange(N_CORE) - np.searchsorted(izs, izs, side="left")
        ggid = binpos // G                            # group within bin
        key = izs.astype(np.int64) * 4096 + ggid      # global (bin, group)
        uniq, ginv = np.unique(key, return_inverse=True)
        n_groups = len(uniq)
        assert n_groups * G <= S, f"padding overflow: {n_groups * G} > {S}"
        slot_in_g = binpos % G
        # group g occupies partition g%128, free blocks (g//128)*G + m
        part = (ginv % P).astype(np.int32)
        free = ((ginv // P) * G + slot_in_g).astype(np.int32)

        # z-row per group, one descriptor per group, j == g ordering
        zrow = np.zeros(S // G, dtype=np.int16)
        zrow[:n_groups] = (uniq // 4096).astype(np.int16)

        # per-slot w / y / x arrays in (partition, free) layout
        w_arr = np.zeros((P, 3, F), dtype=np.float16)
        iy_arr = np.zeros((P, F), dtype=np.int16)
        ix_arr = np.zeros((P, F), dtype=np.int16)
        w_arr[part, 0, free] = wz[order]
        w_arr[part, 1, free] = wy[order]
        w_arr[part, 2, free] = wx[order]
        iy_arr[part, free] = iy[order].astype(np.int16)
        ix_arr[part, free] = ix[order].astype(np.int16)

        in_maps.append({
            "w": w_arr.reshape(P, 3 * F),
            "idxz": wrap16(zrow).reshape(P, F),
            "idxy": wrap16(iy_arr.T.reshape(-1)).reshape(P, F * 8),
            "idxx": wrap16(ix_arr.T.reshape(-1)).reshape(P, F * 8),
            "tbl0": tables[0],
            "tbl1": tables[1],
            "tbl2": tables[2],
        })
        # inverse mapping: sorted order + slot coordinates
        perms.append((order, part, free))
    return in_maps, perms


def _unshard(results, perms):
    outs = []
    for k in range(N_CORES):
        wv = np.asarray(results[k]["out"])           # (P, F)
        order, part, free = perms[k]
        vals = wv[part, free]                        # sorted-point order
        o = np.empty(N_CORE, dtype=np.float32)
        o[order] = vals
        outs.append(o)
    return np.concatenate(outs).reshape(4096, 192).astype(np.float32)


def kernel(in_tensor, line_z, line_y, line_x):
    global _BUILT
    from concourse.bass_utils import run_bass_kernel_spmd

    if _BUILT is None:
        _BUILT = _build_nc()
    nc = _BUILT
    in_maps, perms = _host_prep(np.asarray(in_tensor), np.asarray(line_z),
                                np.asarray(line_y), np.asarray(line_x))
    res = run_bass_kernel_spmd(nc, in_maps, list(range(N_CORES)))
    return _unshard(res.results, perms)


# revision 12
# speedup vs baseline: 1.7258x; 1.2190x over previous
"""CPDecoding (embedding_lookup) Trainium2 kernel, v2.

out[n] = sum_c fz[c,n]*fy[c,n]*fx[c,n], where f* is a 1-D linear
interpolation (grid_sample, align_corners=True) of a (96, 512) line table
at per-point coordinates.

Strategy (8 cores, data-parallel over the N=4096*192 points):
  - Host: compute (i0, w) per point/axis, sort points by z-index and pack
    8 points per z-table row (one 512B gather descriptor serves 8 points),
    pad to a fixed slot count; pack tables as fp16 [f0|delta|pad] 512B rows.
  - Device: dma_gather of one row per (point, y/x-axis) and one row per
    8-point z-group; Act engine materializes per-point weights broadcast
    over components; DVE does interp + products + component reduce; Pool
    does one interp mul + SWDGE descriptor generation.
  - Host: unpermute the per-point sums back to the original order.
"""

import numpy as np

N_CORES = 8
N_TOTAL = 4096 * 192
N_CORE = N_TOTAL // N_CORES      # 98304 points per core
P = 128                          # partitions
G = 8                            # points per z-row group
F = 800                          # free slots per partition (padded)
S = P * F                        # 102400 padded point slots per core
C = 96                           # components
R = 512                          # table resolution
ELEM = 256                       # fp16 elements per z table row (512 bytes)
SS = 64                          # y/x table supersampling factor
SELEM = 128                      # fp16 elements per y/x table row (256 bytes)
CHUNK_F = 32                     # f-blocks per chunk
CHUNK_PTS = P * CHUNK_F          # 4096 points per chunk
N_CHUNKS = F // CHUNK_F          # 25
ZBLK = CHUNK_F // G              # z-row blocks per chunk (4)

_BUILT = None


def _build_nc():
    """Build the per-core Bass program (SPMD, identical on all cores)."""
    import concourse.bacc as bacc
    import concourse.tile as tile
    from concourse import mybir
    from concourse.library_config import mlp as lib_mlp

    dt = mybir.dt
    Alu = mybir.AluOpType
    Axis = mybir.AxisListType

    nc = bacc.Bacc("TRN2", target_bir_lowering=False, debug=False,
                   num_devices=N_CORES, num_swdge_queues=1)

    # host-prepared inputs
    w_d = nc.dram_tensor("w", [P, F], dt.float16, kind="ExternalInput").ap()
    idxz_d = nc.dram_tensor("idxz", [P, F], dt.int16, kind="ExternalInput").ap()
    idxy_d = nc.dram_tensor("idxy", [P, F * 8], dt.int16,
                            kind="ExternalInput").ap()
    idxx_d = nc.dram_tensor("idxx", [P, F * 8], dt.int16,
                            kind="ExternalInput").ap()
    tblz = nc.dram_tensor("tblz", [R, ELEM], dt.float16,
                          kind="ExternalInput").ap()
    tbly = nc.dram_tensor("tbly", [R * SS, SELEM], dt.float16,
                          kind="ExternalInput").ap()
    tblx = nc.dram_tensor("tblx", [R * SS, SELEM], dt.float16,
                          kind="ExternalInput").ap()
    out_d = nc.dram_tensor("out", [P, F], dt.float32, kind="ExternalOutput").ap()

    with tile.TileContext(nc) as tc:
        with tc.tile_pool(name="persist", bufs=1) as pp:
            w_all = pp.tile([P, F], dt.float16, tag="w")
            nc.sync.dma_start(w_all[:], w_d)
            idx_z = pp.tile([P, F], dt.int16, tag="iz")
            nc.sync.dma_start(idx_z[:], idxz_d)
            idx_y = pp.tile([P, F * 8], dt.int16, tag="iy")
            nc.sync.dma_start(idx_y[:], idxy_d)
            idx_x = pp.tile([P, F * 8], dt.int16, tag="ix")
            nc.sync.dma_start(idx_x[:], idxx_d)
            out_full = pp.tile([P, F], dt.float32, tag="out")

            with (
                tc.tile_pool(name="gath", bufs=2) as gp,
                tc.tile_pool(name="wt", bufs=2) as wp,
                tc.tile_pool(name="mid", bufs=2) as mp,
            ):
                with tc.tile_critical():
                    nc.gpsimd.load_library(lib_mlp)
                for c in range(N_CHUNKS):
                    fs = CHUNK_F * c

                    # --- gathers (rows: [f0(96) | delta(96) | pad]) ---
                    # idx tiles are wrapped-16 ([16, n/16] per band, replicated
                    # to all 8 bands); chunk c uses its 16-wrapped column slice
                    zc = ZBLK * P // 16              # 32 idx cols per chunk
                    gz = gp.tile([P, ZBLK, ELEM], dt.float16, tag="gz")
                    nc.gpsimd.dma_gather(
                        gz[:], tblz, idx_z[:, zc * c:zc * (c + 1)],
                        ZBLK * P, ZBLK * P, ELEM, elem_step=ELEM,
                        queue_num=0, single_packet=False)
                    yc = CHUNK_PTS // 16             # 256 idx cols per chunk
                    gy = gp.tile([P, CHUNK_F, SELEM], dt.float16, tag="gy")
                    nc.gpsimd.dma_gather(
                        gy[:], tbly, idx_y[:, yc * c:yc * (c + 1)],
                        CHUNK_PTS, CHUNK_PTS, SELEM, elem_step=SELEM,
                        queue_num=0, single_packet=False)
                    gx = gp.tile([P, CHUNK_F, SELEM], dt.float16, tag="gx")
                    nc.gpsimd.dma_gather(
                        gx[:], tblx, idx_x[:, yc * c:yc * (c + 1)],
                        CHUNK_PTS, CHUNK_PTS, SELEM, elem_step=SELEM,
                        queue_num=0, single_packet=False)

                    # --- z weight broadcast tile (Act engine) ---
                    wtz = wp.tile([P, CHUNK_F, C], dt.float16, tag="wtz")
                    src = (w_all[:, fs:fs + CHUNK_F]
                           .unsqueeze(2).broadcast_to([P, CHUNK_F, C]))
                    nc.scalar.copy(wtz[:], src)

                    # --- z interp (rows shared by groups of 8 points) ---
                    d_z = (gz[:, :, C:2 * C].unsqueeze(2)
                           .broadcast_to([P, ZBLK, G, C]))
                    f0_z = (gz[:, :, 0:C].unsqueeze(2)
                            .broadcast_to([P, ZBLK, G, C]))
                    fz = mp.tile([P, CHUNK_F, C], dt.float16, tag="fz")
                    fz4 = fz[:].rearrange("p (q g) v -> p q g v", g=G)
                    wt04 = wtz[:].rearrange("p (q g) v -> p q g v", g=G)
                    nc.vector.tensor_mul(fz4, d_z, wt04)
                    nc.vector.tensor_add(fz4, fz4, f0_z)

                    # --- products with supersampled y/x rows (DVE) ---
                    nc.vector.tensor_mul(fz[:], fz[:], gy[:, :, 0:C])
                    nc.vector.tensor_mul(fz[:], fz[:], gx[:, :, 0:C])
                    # binary-tree halvings at tensor_tensor 2x rate, then a
                    # short tensor_reduce tail (reduce gets no DVE perf mode)
                    half = C
                    while half >= 12:
                        half //= 2
                        nc.vector.tensor_add(fz[:, :, 0:half],
                                             fz[:, :, 0:half],
                                             fz[:, :, half:2 * half])
                    nc.vector.reduce_sum(out_full[:, fs:fs + CHUNK_F],
                                         fz[:, :, 0:half], axis=Axis.X)

                nc.sync.dma_start(out_d, out_full[:])

    nc.compile()
    return nc


def _host_prep(in_tensor, line_z, line_y, line_x):
    """Build per-core input maps; returns (in_maps, per-core unsort perms)."""
    pts = np.ascontiguousarray(in_tensor.reshape(-1, 3).astype(np.float32))

    # z table: [f0(96) | delta(96) | pad] rows of 512B
    Lz = np.asarray(line_z, dtype=np.float32)
    z0 = Lz.T                                        # (512, 96)
    z1 = np.concatenate([Lz.T[1:], Lz.T[-1:]], axis=0)
    tbl_z = np.zeros((R, ELEM), dtype=np.float16)
    tbl_z[:, 0:C] = z0.astype(np.float16)
    tbl_z[:, C:2 * C] = (z1 - z0).astype(np.float16)

    # y/x tables: 64x supersampled, interpolation baked in, f0-only 256B rows
    def supersample(L):
        Lf = np.asarray(L, dtype=np.float32).T       # (512, 96)
        f0 = Lf
        f1 = np.concatenate([Lf[1:], Lf[-1:]], axis=0)
        r = (np.arange(SS, dtype=np.float32) / SS)[None, :, None]
        fine = f0[:, None, :] * (1.0 - r) + f1[:, None, :] * r
        row = np.zeros((R * SS, SELEM), dtype=np.float16)
        row[:, 0:C] = fine.reshape(R * SS, C).astype(np.float16)
        return row
    tbl_y = supersample(line_y)
    tbl_x = supersample(line_x)

    # per-point indices/weights, axes ordered [z, y, x] = cols [2, 1, 0]
    pos = (pts + 1.0) * 0.5 * (R - 1)
    i0 = np.clip(np.floor(pos), 0, R - 1).astype(np.int32)
    w = (pos - i0).astype(np.float16)
    # supersampled y/x indices (nearest of the 64x grid)
    isup = np.clip(np.round(pos * SS), 0, (R - 1) * SS).astype(np.int32)

    def wrap16(flat):
        """j-ordered descriptor index list -> [16, n/16] band, replicated
        to all 8 16-partition bands."""
        w16 = flat.reshape(-1, 16).T
        return np.ascontiguousarray(np.tile(w16, (8, 1)))

    in_maps = []
    perms = []
    for k in range(N_CORES):
        sl = slice(k * N_CORE, (k + 1) * N_CORE)
        iz = i0[sl, 2]
        iy, ix = isup[sl, 1], isup[sl, 0]
        wz = w[sl, 2]

        # sort by z-index; emit fixed-size groups of G per z-bin (padded)
        order = np.argsort(iz, kind="stable")
        izs = iz[order]
        # position of each sorted point within its z-bin
        binpos = np.arange(N_CORE) - np.searchsorted(izs, izs, side="left")
        ggid = binpos // G                            # group within bin
        key = izs.astype(np.int64) * 4096 + ggid      # global (bin, group)
        uniq, ginv = np.unique(key, return_inverse=True)
        n_groups = len(uniq)
        assert n_groups * G <= S, f"padding overflow: {n_groups * G} > {S}"
        slot_in_g = binpos % G
        # group g occupies partition g%128, free blocks (g//128)*G + m
        part = (ginv % P).astype(np.int32)
        free = ((ginv // P) * G + slot_in_g).astype(np.int32)

        # z-row per group, one descriptor per group, j == g ordering
        zrow = np.zeros(S // G, dtype=np.int16)
        zrow[:n_groups] = (uniq // 4096).astype(np.int16)

        # per-slot w / y / x arrays in (partition, free) layout
        w_arr = np.zeros((P, F), dtype=np.float16)
        iy_arr = np.zeros((P, F), dtype=np.int16)
        ix_arr = np.zeros((P, F), dtype=np.int16)
        w_arr[part, free] = wz[order]
        iy_arr[part, free] = iy[order].astype(np.int16)
        ix_arr[part, free] = ix[order].astype(np.int16)

        in_maps.append({
            "w": w_arr,
            "idxz": wrap16(zrow).reshape(P, F),
            "idxy": wrap16(iy_arr.T.reshape(-1)).reshape(P, F * 8),
            "idxx": wrap16(ix_arr.T.reshape(-1)).reshape(P, F * 8),
            "tblz": tbl_z,
            "tbly": tbl_y,
            "tblx": tbl_x,
        })
        # inverse mapping: sorted order + slot coordinates
        perms.append((order, part, free))
    return in_maps, perms


def _unshard(results, perms):
    outs = []
    for k in range(N_CORES):
        wv = np.asarray(results[k]["out"])           # (P, F)
        order, part, free = perms[k]
        vals = wv[part, free]                        # sorted-point order
        o = np.empty(N_CORE, dtype=np.float32)
        o[order] = vals
        outs.append(o)
    return np.concatenate(outs).reshape(4096, 192).astype(np.float32)


def kernel(in_tensor, line_z, line_y, line_x):
    global _BUILT
    from concourse.bass_utils import run_bass_kernel_spmd

    if _BUILT is None:
        _BUILT = _build_nc()
    nc = _BUILT
    in_maps, perms = _host_prep(np.asarray(in_tensor), np.asarray(line_z),
                                np.asarray(line_y), np.asarray(line_x))
    res = run_bass_kernel_spmd(nc, in_maps, list(range(N_CORES)))
    return _unshard(res.results, perms)
